# revision 1
# baseline (speedup 1.0000x reference)
"""Trainium2 Bass kernel for nn_AdaConvNeXt (moe_routing).

Strategy (data-parallel over batch, 16 images per NeuronCore, fully local):
  - Depthwise 7x7 conv split across TensorE (diagonal-weight matmuls
    accumulated in PSUM, valid-window per tap => no padding) and VectorE
    (tensor_scalar mult @4x + tensor_tensor add @2x, even-dx taps only so
    the bf16 packed perf modes stay aligned).
  - LayerNorm stats via ones-vector matmuls on TensorE; per-token
    mu/istd math batched over 4 images on (4,784) rows (sqrt on ScalarE,
    exact reciprocal on VectorE); broadcast back over partitions with
    K=1 matmuls.
  - Both routed branches are computed for all tokens; the routing
    gather/scatter is realized exactly with 0/1 masks (idx2-wins
    collision semantics preserved), host-precomputed from idx1/idx2 and
    broadcast on-device. All affine weights are pre-folded on the host
    (LN affine into w1/fp_w, gamma into w2/fp_w, biases into GELU bias
    and per-channel constants).
  - FFN/fast-path matmuls in bf16 (output error is ~1e-7 of |x| since
    gamma=1e-6 and the residual passes through in f32).
"""

import os
import numpy as np
import ml_dtypes

import concourse.bass as bass
import concourse.bacc as bacc
import concourse.mybir as mybir
import concourse.tile as tile
from concourse.bass_utils import run_bass_kernel_spmd
from concourse import dve_ops as _dve_ops
from concourse.dve_spec import Spec as _Spec, Src0 as _Src0, Src1 as _Src1, \
    C0 as _C0, C1 as _C1, lower as _dve_lower, _has_src1 as _dve_has_src1
from concourse.dve_uop import DveOpSpec as _DveOpSpec


def _register_pair_mac():
    """Runtime-register a fused 2-tap conv MAC: out = in0*s0 + in1*s1."""
    name = "PAIR_MAC_ANT_K"
    for o in _dve_ops.OPS:
        if o.name == name:
            return o
    import numpy as _np
    spec = _Spec(
        body=_Src0 * _C0 + _Src1 * _C1,
        reference=lambda in0, in1, s0, s1, imm2:
            in0.astype(_np.float32) * s0 + in1 * s1,
    )
    row = _dve_ops._CUSTOM_DVE_ROW_BASE + len(_dve_ops.OPS)
    shas = {}
    for ver in ("v3", "v4"):
        try:
            uops = _dve_lower(spec, ver=ver)
            shas[ver] = _DveOpSpec(
                name=name, opcode=row, uops=uops,
                rd1_en=_dve_has_src1(spec)).sha(ver)
        except Exception:
            pass
    op = _dve_ops.DveOp(name, spec, subdim=False, uops_sha=shas)
    _dve_ops.OPS.append(op)
    _dve_ops.CUSTOM_DVE_SPECS[name] = spec
    _dve_ops._SUB_OPCODE_FOR_NAME[name] = row
    return op


PAIR_MAC = _register_pair_mac()

BF16 = mybir.dt.bfloat16
FP8 = mybir.dt.float8e4
W8SCALE = 64.0
F32 = mybir.dt.float32
ADD = mybir.AluOpType.add
MULT = mybir.AluOpType.mult
AF = mybir.ActivationFunctionType

N_CORES = 8
B, C, H, W = 128, 384, 28, 28
N = H * W          # 784
BL = B // N_CORES  # 16 images per core
NG = C // 128      # 3 channel groups
FG = (4 * C) // 128  # 12 ffn groups
HALF = N // 2      # 392 (one PSUM bank of f32, = 14 rows)
EPS = 1e-6
STAT_BLK = 2       # images per batched-stats block (partitions 0 and 32)

# --- tap split ---------------------------------------------------------------
_ALL_TAPS = [(dy, dx) for dy in range(-3, 4) for dx in range(-3, 4)]
# VectorE taps must have even dx so bf16 2x/4x alignment rules hold
# (offsets/lengths even). (0,0) must stay on PE: it is the only tap whose
# window covers the full PSUM region, so it is the start=True matmul.
_EVEN = [(dy, dx) for dy in range(-3, 4) for dx in (-2, 0, 2) if (dy, dx) != (0, 0)]
_EVEN.sort(key=lambda t: (28 - abs(t[0])) * (28 - abs(t[1])))
DVE_TAPS = _EVEN[:8]        # product (tensor_scalar) + add, both on VectorE
SC_TAPS = _EVEN[8:]         # product on ScalarE (per-partition scale AP), add on VectorE
NONPE = DVE_TAPS + SC_TAPS
PE_TAPS = [t for t in _ALL_TAPS if t not in NONPE]
PE_TAPS.remove((0, 0))
PE_TAPS.insert(0, (0, 0))  # start=True tap first
NPE = len(PE_TAPS)
NDVE = len(NONPE)


def _win(dy, dx):
    """Valid output window of a tap: rows [r0,r1), cols [c0,c1)."""
    return max(0, -dy), H - max(0, dy), max(0, -dx), W - max(0, dx)


def build_bass(BL_=BL):
    nc = bacc.Bacc(None, target_bir_lowering=False, debug=False)

    x_d = nc.declare_dram_parameter("x", [BL_, C, H, W], F32, isOutput=False)
    masks_d = nc.declare_dram_parameter("masks", [2, BL_, H, W], BF16, isOutput=False)
    diag_d = nc.declare_dram_parameter("diag", [128, NG, NPE, 128], BF16, isOutput=False)
    w1t_d = nc.declare_dram_parameter("w1t", [128, NG, FG, 128], BF16, isOutput=False)
    w2ft_d = nc.declare_dram_parameter("w2ft", [128, FG, NG, 128], BF16, isOutput=False)
    w2qt_d = nc.declare_dram_parameter("w2qt", [128, NG, NG, 128], BF16, isOutput=False)
    dwtap_d = nc.declare_dram_parameter("dwtap", [128, NG, NDVE], F32, isOutput=False)
    # cvec columns: 0..2 dw_b[g], 3..14 c1[fg], 15..17 c1out[og], 18..20 c2[og]
    cvec_d = nc.declare_dram_parameter("cvec", [128, 3 + FG + 3 + 3], F32, isOutput=False)
    out_d = nc.declare_dram_parameter("out", [BL_, C, H, W], F32, isOutput=True)

    from contextlib import ExitStack
    with ExitStack() as es:
        tc = es.enter_context(tile.TileContext(nc))
        pool = lambda name, bufs, **kw: es.enter_context(
            tc.tile_pool(name=name, bufs=bufs, **kw))
        cpool = pool("consts", 1)
        xin_pool = pool("xin", 2)
        xres_pool = pool("xres", 2)
        acc_pool = pool("acc", 2)
        y_pool = pool("ybuf", STAT_BLK + 2)
        ysq_pool = pool("ysq", 2)
        z_pool = pool("zbuf", 2)
        g_pool = pool("gbuf", 2)
        mb_pool = pool("maskb", 2)
        tmp_pool = pool("tmp", 2)
        tmps_pool = pool("tmpsq", 4)
        tsm_pool = pool("tsm", 2)
        rows_pool = pool("rows", 1)
        o_pool = pool("obuf", 2)
        dram_pool = pool("dscratch", 2, space=bass.MemorySpace.DRAM)
        py_pool = pool("py", 2, space=bass.MemorySpace.PSUM)
        ph_pool = pool("ph", 3, space=bass.MemorySpace.PSUM)
        paux_pool = pool("paux", 1, space=bass.MemorySpace.PSUM)
        ppq_pool = pool("ppq", 2, space=bass.MemorySpace.PSUM)
        if True:
            # ---- constants into SBUF ----
            diag_sb = cpool.tile([128, NG, NPE, 128], BF16)
            for _g in range(NG):
                nc.sync.dma_start(diag_sb[:, _g], diag_d[:, _g])
            w1t_sb = cpool.tile([128, NG, FG, 128], BF16)
            nc.sync.dma_start(w1t_sb[:], w1t_d[:])
            w2ft_sb = cpool.tile([128, FG, NG, 128], BF16)
            nc.sync.dma_start(w2ft_sb[:], w2ft_d[:])
            w2qt_sb = cpool.tile([128, NG, NG, 128], BF16)
            nc.sync.dma_start(w2qt_sb[:], w2qt_d[:])
            dwtap_sb = cpool.tile([128, NG, NDVE], F32)
            nc.sync.dma_start(dwtap_sb[:], dwtap_d[:])
            cvec_sb = cpool.tile([128, 3 + FG + 3 + 3], F32)
            nc.sync.dma_start(cvec_sb[:], cvec_d[:])

            ones_col = cpool.tile([128, 1], BF16)      # stats lhsT
            nc.vector.memset(ones_col[:], 1.0)
            eps_col = cpool.tile([33, 1], F32)         # LN eps as bias AP
            nc.vector.memset(eps_col[:], EPS)

            n_blocks = (BL_ + STAT_BLK - 1) // STAT_BLK
            for blk in range(n_blocks):
                imgs = list(range(blk * STAT_BLK, min((blk + 1) * STAT_BLK, BL_)))
                nb = len(imgs)

                # per-image rows of per-token stats (image ii at partition 32*ii)
                srow = rows_pool.tile([33, N], F32)  # sum/C
                qrow = rows_pool.tile([33, N], F32)  # sumsq/C

                y_tiles = {}
                for ii, img in enumerate(imgs):
                    # ---- load input (bf16 cast via SWDGE) ----
                    x_bf = xin_pool.tile([128, NG, H, W], BF16)
                    for g in range(NG):
                        nc.gpsimd.dma_start(
                            out=x_bf[:, g], in_=x_d[img, g * 128:(g + 1) * 128])

                    # ---- depthwise conv ----
                    acc = acc_pool.tile([128, NG, H, W], BF16)
                    nc.gpsimd.memset(acc[:], 0.0)
                    tmp = tmp_pool.tile([128, H, W], BF16, tag="tmp")
                    y_bf = y_pool.tile([128, NG, H, W], BF16)
                    y_tiles[img] = y_bf
                    ysq = ysq_pool.tile([128, NG, H, W], BF16)

                    for g in range(NG):
                        # ScalarE-product taps: per-channel scale on ACT,
                        # accumulate on VectorE (emitted first so ACT races ahead)
                        tmps = [tmps_pool.tile([128, H, W], BF16, tag="tmps", name=f"tmps{_i}") for _i in range(4)]
                        for j, (dy, dx) in enumerate(SC_TAPS):
                            k = len(DVE_TAPS) + j
                            r0, r1, c0, c1 = _win(dy, dx)
                            tbuf = tmps[j % 4]
                            nc.scalar.activation(
                                tbuf[:, r0:r1, c0:c1],
                                x_bf[:, g, r0 + dy:r1 + dy, c0 + dx:c1 + dx],
                                AF.Copy, scale=dwtap_sb[:, g, k:k + 1])
                            nc.vector.tensor_tensor(
                                out=acc[:, g, r0:r1, c0:c1],
                                in0=acc[:, g, r0:r1, c0:c1],
                                in1=tbuf[:, r0:r1, c0:c1], op=ADD)
                        # VectorE taps: product @2x + accumulate @2x
                        for k, (dy, dx) in enumerate(DVE_TAPS):
                            r0, r1, c0, c1 = _win(dy, dx)
                            nc.vector.tensor_scalar(
                                out=tmp[:, r0:r1, c0:c1],
                                in0=x_bf[:, g, r0 + dy:r1 + dy, c0 + dx:c1 + dx],
                                scalar1=dwtap_sb[:, g, k:k + 1],
                                scalar2=None, op0=MULT)
                            nc.vector.tensor_tensor(
                                out=acc[:, g, r0:r1, c0:c1],
                                in0=acc[:, g, r0:r1, c0:c1],
                                in1=tmp[:, r0:r1, c0:c1], op=ADD)
                        # TensorE taps: tap-major over both PSUM half-banks
                        psum_y = [py_pool.tile([128, 14, W], F32, tag="py", name=f"psum_y{_hb}")
                                  for _hb in range(2)]
                        for ti, (dy, dx) in enumerate(PE_TAPS):
                            r0, r1, c0, c1 = _win(dy, dx)
                            for hb in range(2):
                                hr0, hr1 = 14 * hb, 14 * hb + 14
                                rr0, rr1 = max(r0, hr0), min(r1, hr1)
                                if rr0 >= rr1:
                                    continue
                                nc.tensor.matmul(
                                    psum_y[hb][:, rr0 - hr0:rr1 - hr0, c0:c1],
                                    diag_sb[:, g, ti, :],
                                    x_bf[:, g, rr0 + dy:rr1 + dy, c0 + dx:c1 + dx],
                                    start=(ti == 0), stop=(ti == NPE - 1),
                                    skip_group_check=True)
                        for hb in range(2):
                            hr0, hr1 = 14 * hb, 14 * hb + 14
                            # y = (acc + dw_b) + psum_y
                            nc.vector.scalar_tensor_tensor(
                                out=y_bf[:, g, hr0:hr1, :],
                                in0=acc[:, g, hr0:hr1, :],
                                scalar=cvec_sb[:, g:g + 1],
                                in1=psum_y[hb][:],
                                op0=ADD, op1=ADD)
                        # y^2 for variance
                        nc.scalar.activation(ysq[:, g], y_bf[:, g], AF.Square)

                    # ---- LN stats: per-token sum and sumsq via ones-matmuls ----
                    for lam in range(2):
                        rs = slice(14 * lam, 14 * lam + 14)
                        pst = paux_pool.tile([33, HALF], F32, tag='aux')
                        for g in range(NG):
                            nc.tensor.matmul(
                                pst[0:1, :], ones_col[:], y_bf[:, g, rs, :],
                                start=(g == 0), stop=(g == NG - 1),
                                skip_group_check=True)
                        for g in range(NG):
                            nc.tensor.matmul(
                                pst[32:33, :], ones_col[:], ysq[:, g, rs, :],
                                start=(g == 0), stop=(g == NG - 1),
                                tile_position=(0, 32),
                                skip_group_check=True)
                        cs = slice(HALF * lam, HALF * lam + HALF)
                        ps = 32 * ii
                        nc.scalar.activation(
                            srow[ps:ps + 1, cs], pst[0:1, :], AF.Copy, scale=1.0 / C)
                        nc.scalar.activation(
                            qrow[ps:ps + 1, cs], pst[32:33, :], AF.Copy, scale=1.0 / C)

                # ---- batched stats math over the block: istd, -mu*istd ----
                np_ = 32 * (nb - 1) + 1  # partitions actually used
                musq = rows_pool.tile([33, N], F32, tag="rowsw1")
                nc.vector.tensor_tensor(out=musq[:np_], in0=srow[:np_], in1=srow[:np_], op=MULT)
                veps = rows_pool.tile([33, N], F32, tag="rowsw2")
                nc.vector.scalar_tensor_tensor(
                    out=veps[:np_], in0=musq[:np_], scalar=-1.0, in1=qrow[:np_],
                    op0=MULT, op1=ADD)
                sd = rows_pool.tile([33, N], F32, tag="rowsw1")
                nc.scalar.activation(sd[:np_], veps[:np_], AF.Sqrt, bias=eps_col[:np_])
                istd_r = rows_pool.tile([33, N], F32)
                with nc.allow_low_precision(reason="istd in bf16 is plenty (gamma=1e-6)"):
                    nc.vector.reciprocal_approx_fast(out=istd_r[:np_], in_=sd[:np_])
                nmi_r = rows_pool.tile([33, N], F32, tag="rowsw2")
                nc.vector.scalar_tensor_tensor(
                    out=nmi_r[:np_], in0=srow[:np_], scalar=-1.0, in1=istd_r[:np_],
                    op0=MULT, op1=MULT)

                # stage the per-image stat rows in DRAM so they can be
                # partition-broadcast by DMA (SBUF sources cannot)
                stat_dr = {}
                for ii, img in enumerate(imgs):
                    ps = 32 * ii
                    sc = dram_pool.tile([2, N], F32, tag="sc", name=f"sc{ii}")
                    nc.sync.dma_start(out=sc[0:1, :], in_=istd_r[ps:ps + 1, :])
                    nc.sync.dma_start(out=sc[1:2, :], in_=nmi_r[ps:ps + 1, :])
                    stat_dr[img] = sc

                # ---- phase 2: z, FFN, merge, store ----
                for ii, img in enumerate(imgs):
                    y_bf = y_tiles[img]
                    sc = stat_dr[img]
                    x_f32 = xres_pool.tile([128, NG, H, W], F32)
                    for g in range(NG):
                        nc.sync.dma_start(
                            out=x_f32[:, g], in_=x_d[img, g * 128:(g + 1) * 128])
                    m1b = mb_pool.tile([128, H, W], BF16, tag="m1b")
                    nc.sync.dma_start(
                        out=m1b[:], in_=masks_d[0:1, img].partition_broadcast(128))
                    m2b = mb_pool.tile([128, H, W], BF16, tag="m2b")
                    nc.sync.dma_start(
                        out=m2b[:], in_=masks_d[1:2, img].partition_broadcast(128))
                    istd_b = mb_pool.tile([128, H, W], BF16, tag="istdb")
                    nc.gpsimd.dma_start(
                        out=istd_b[:], in_=sc[0:1, :].partition_broadcast(128))
                    nmi_b = mb_pool.tile([128, H, W], BF16, tag="nmib")
                    nc.gpsimd.dma_start(
                        out=nmi_b[:], in_=sc[1:2, :].partition_broadcast(128))

                    # z = y * istd - mu*istd  (bf16 SBUF, 2x mode)
                    z_bf = z_pool.tile([128, NG, H, W], BF16)
                    for g in range(NG):
                        tz = tmp_pool.tile([128, H, W], BF16, tag="tz")
                        nc.vector.tensor_tensor(
                            out=tz[:], in0=y_bf[:, g], in1=istd_b[:], op=MULT)
                        nc.vector.tensor_tensor(
                            out=z_bf[:, g], in0=tz[:], in1=nmi_b[:], op=ADD)

                    # FFN: h = W1^T z (+c1), gelu, p = W2f^T g; q = W2q^T z
                    g_sb = g_pool.tile([128, FG, H, W], BF16)
                    for fg in range(FG):
                        for lam in range(2):
                            rs = slice(14 * lam, 14 * lam + 14)
                            ph = ph_pool.tile([128, 14, W], F32, tag="ph")
                            for cg in range(NG):
                                nc.tensor.matmul(
                                    ph[:], w1t_sb[:, cg, fg, :], z_bf[:, cg, rs, :],
                                    start=(cg == 0), stop=(cg == NG - 1))
                            nc.scalar.activation(
                                g_sb[:, fg, rs, :], ph[:], AF.Gelu,
                                bias=cvec_sb[:, 3 + fg:4 + fg])
                    for og in range(NG):
                        rs2 = [slice(0, 14), slice(14, 28)]
                        pp = [ppq_pool.tile([128, 14, W], F32, tag="pq", name=f"pp{_l}")
                              for _l in range(2)]
                        for fg in range(FG):
                            for lam in range(2):
                                nc.tensor.matmul(
                                    pp[lam][:], w2ft_sb[:, fg, og, :],
                                    g_sb[:, fg, rs2[lam], :],
                                    start=(fg == 0), stop=(fg == FG - 1))
                        t1p = [tsm_pool.tile([128, 14, W], BF16, tag="t1p", name=f"t1p{_l}")
                               for _l in range(2)]
                        for lam in range(2):
                            nc.scalar.activation(
                                t1p[lam][:], pp[lam][:], AF.Identity,
                                bias=cvec_sb[:, 15 + og:16 + og])
                        pq = [ppq_pool.tile([128, 14, W], F32, tag="pq", name=f"pq{_l}")
                              for _l in range(2)]
                        for cg in range(NG):
                            for lam in range(2):
                                nc.tensor.matmul(
                                    pq[lam][:], w2qt_sb[:, cg, og, :],
                                    z_bf[:, cg, rs2[lam], :],
                                    start=(cg == 0), stop=(cg == NG - 1))
                        t2p = [tsm_pool.tile([128, 14, W], BF16, tag="t2p", name=f"t2p{_l}")
                               for _l in range(2)]
                        for lam in range(2):
                            nc.scalar.activation(
                                t2p[lam][:], pq[lam][:], AF.Identity,
                                bias=cvec_sb[:, 18 + og:19 + og])
                        for lam in range(2):
                            rs = rs2[lam]
                            t1 = tsm_pool.tile([128, 14, W], BF16, tag="t1")
                            nc.vector.tensor_tensor(
                                out=t1[:], in0=t1p[lam][:], in1=m1b[:, rs, :], op=MULT)
                            t2 = tsm_pool.tile([128, 14, W], BF16, tag="t2")
                            nc.vector.tensor_tensor(
                                out=t2[:], in0=t2p[lam][:], in1=m2b[:, rs, :], op=MULT)
                            s12 = tsm_pool.tile([128, 14, W], BF16, tag="s12")
                            nc.gpsimd.tensor_tensor(
                                out=s12[:], in0=t1[:], in1=t2[:], op=ADD)
                            ox = o_pool.tile([128, 14, W], F32)
                            nc.gpsimd.tensor_tensor(
                                out=ox[:], in0=x_f32[:, og, rs, :], in1=s12[:], op=ADD)
                            nc.sync.dma_start(
                                out=out_d[img, og * 128:(og + 1) * 128, rs, :], in_=ox[:])
    nc.compile()
    return nc


# ---------------------------------------------------------------------------
# host side
# ---------------------------------------------------------------------------

def _fold_host(inputs):
    f32 = np.float32
    bf16 = ml_dtypes.bfloat16
    dw_w = np.asarray(inputs["dw_w"], f32)      # (C,1,7,7)
    dw_b = np.asarray(inputs["dw_b"], f32)
    norm_w = np.asarray(inputs["norm_w"], f32)
    norm_b = np.asarray(inputs["norm_b"], f32)
    w1 = np.asarray(inputs["w1"], f32)
    b1 = np.asarray(inputs["b1"], f32)
    w2 = np.asarray(inputs["w2"], f32)
    b2 = np.asarray(inputs["b2"], f32)
    gamma = np.asarray(inputs["gamma"], f32)
    fp_norm_w = np.asarray(inputs["fp_norm_w"], f32)
    fp_norm_b = np.asarray(inputs["fp_norm_b"], f32)
    fp_w = np.asarray(inputs["fp_w"], f32)
    fp_b = np.asarray(inputs["fp_b"], f32)
    fp_gamma = np.asarray(inputs["fp_gamma"], f32)

    W1 = norm_w[:, None] * w1                     # (C, 4C)
    c1 = norm_b @ w1 + b1                         # (4C,)
    W2f = w2 * gamma[None, :]                     # (4C, C)
    c1out = b2 * gamma                            # (C,)
    W2q = (fp_norm_w[:, None] * fp_w) * fp_gamma[None, :]  # (C, C)
    c2 = (fp_norm_b @ fp_w + fp_b) * fp_gamma     # (C,)

    # diag tiles: [c, g, k, m] = w[g*128+c, tap_k] if m == c
    diag = np.zeros((128, NG, NPE, 128), f32)
    for g in range(NG):
        for k, (dy, dx) in enumerate(PE_TAPS):
            wt = dw_w[g * 128:(g + 1) * 128, 0, dy + 3, dx + 3]
            diag[np.arange(128), g, k, np.arange(128)] = wt
    dwtap = np.zeros((128, NG, NDVE), f32)
    for g in range(NG):
        for k, (dy, dx) in enumerate(NONPE):
            dwtap[:, g, k] = dw_w[g * 128:(g + 1) * 128, 0, dy + 3, dx + 3]

    w1t = np.zeros((128, NG, FG, 128), f32)
    for cg in range(NG):
        for fg in range(FG):
            w1t[:, cg, fg, :] = W1[cg * 128:(cg + 1) * 128, fg * 128:(fg + 1) * 128]
    w2ft = np.zeros((128, FG, NG, 128), f32)
    for fg in range(FG):
        for og in range(NG):
            w2ft[:, fg, og, :] = W2f[fg * 128:(fg + 1) * 128, og * 128:(og + 1) * 128]
    w2qt = np.zeros((128, NG, NG, 128), f32)
    for cg in range(NG):
        for og in range(NG):
            w2qt[:, cg, og, :] = W2q[cg * 128:(cg + 1) * 128, og * 128:(og + 1) * 128]

    cvec = np.zeros((128, 3 + FG + 3 + 3), f32)
    for g in range(NG):
        cvec[:, g] = dw_b[g * 128:(g + 1) * 128]
    for fg in range(FG):
        cvec[:, 3 + fg] = c1[fg * 128:(fg + 1) * 128]
    for og in range(NG):
        cvec[:, 15 + og] = c1out[og * 128:(og + 1) * 128]
        cvec[:, 18 + og] = c2[og * 128:(og + 1) * 128]

    return dict(
        diag=diag.astype(bf16),
        w1t=w1t.astype(bf16),
        w2ft=w2ft.astype(bf16),
        w2qt=w2qt.astype(bf16),
        dwtap=dwtap,
        cvec=cvec,
    )


def _masks_host(idx1, idx2, Bn):
    m2 = np.zeros((Bn, N), np.float32)
    np.put_along_axis(m2, np.asarray(idx2, np.int64), 1.0, axis=1)
    m1 = np.zeros((Bn, N), np.float32)
    np.put_along_axis(m1, np.asarray(idx1, np.int64), 1.0, axis=1)
    m1 = m1 * (1.0 - m2)  # reference scatter order: idx2 wins collisions
    return m1.astype(ml_dtypes.bfloat16), m2.astype(ml_dtypes.bfloat16)


LAST_RESULT = None


def kernel(**inputs):
    global LAST_RESULT
    x = np.ascontiguousarray(np.asarray(inputs["x"], np.float32))
    Bn = x.shape[0]
    bl = Bn // N_CORES
    assert Bn % N_CORES == 0

    folded = _fold_host(inputs)
    m1, m2 = _masks_host(inputs["idx1"], inputs["idx2"], Bn)

    nc = build_bass(bl)

    in_maps = []
    for c in range(N_CORES):
        sl = slice(c * bl, (c + 1) * bl)
        masks = np.stack([
            m1[sl].reshape(bl, H, W), m2[sl].reshape(bl, H, W)], axis=0)
        in_maps.append(dict(
            x=x[sl],
            masks=np.ascontiguousarray(masks),
            **folded,
        ))

    trace = bool(int(os.environ.get("BASS_KERNEL_TRACE", "0")))
    res = run_bass_kernel_spmd(nc, in_maps, list(range(N_CORES)), trace=trace)
    LAST_RESULT = res
    out = np.concatenate([res.results[c]["out"] for c in range(N_CORES)], axis=0)
    return out

